# revision 25
# baseline (speedup 1.0000x reference)
"""LoRA linear kernel for Trainium2, SPMD across 8 NeuronCores.

Computes out = x @ W.T + bias + (x @ A.T) @ B.T * (alpha/rank) for
x:[4,2048,4096], W:[4096,4096], bias:[4096], A:[16,4096], B:[4096,16].

The rank-16 LoRA delta is folded on the host (W_eff = W + scale*B@A, the
standard merged-LoRA inference form), so the device runs a single dense
GEMM + bias. Sharding: data-parallel over tokens. Each core takes 1024
tokens and computes all 4096 output features. The host pre-transposes x
and W_eff so the contraction dim lands on the SBUF partition axis; each
core computes out.T for its token shard and the host transposes back.

Operands are bf16 (fp32 PSUM accumulation): unlike fp32r, bf16 matmuls
take a separate LDWEIGHTS that the PE's reorder window hoists behind the
previous matmul's streaming, so the stationary-load cost vanishes. Bias
is added by the DVE during the PSUM->SBUF drain (per-partition scalar).

DMA layout is tuned for descriptor size (per-SDMA-engine throughput is
overhead-bound for small lines): W_eff is pre-arranged on the host so
each 128-output-feature chunk is contiguous per partition (8KB
descriptors on the scalar ring); x streams as full-row k-chunk tiles
(2KB descriptors) on the sync ring; outputs also go out on the sync
ring, which is idle once x has landed, so they never queue behind W.
A 4-group staggered k-major prologue keeps the PE fed while x streams.
"""

import sys
import types

import numpy as np

_REPO = "/opt/trn_rl_repo"
if _REPO not in sys.path:
    sys.path.insert(0, _REPO)

import ml_dtypes  # noqa: E402

import concourse.bass as bass  # noqa: E402
import concourse.mybir as mybir  # noqa: E402
import concourse.tile as tile  # noqa: E402

F32 = mybir.dt.float32
BF16 = mybir.dt.bfloat16
BF16_NP = ml_dtypes.bfloat16

B_BATCH, SEQ, DIN = 4, 2048, 4096
DOUT = 4096
RANK = 16
SCALE = 1.0 / 16.0
N_CORES = 8
TOK = B_BATCH * SEQ  # 8192
TOK_C = TOK // N_CORES  # 1024 tokens per core
KC = DIN // 128  # 32 contraction chunks
NC_OUT = DOUT // 128  # 32 output-feature chunks per core
TBLK = 512  # moving free dim per matmul (one PSUM bank)
NT = TOK_C // TBLK  # 2 token blocks per core


def _install_ntff_hook():
    """Best-effort shim so trace=True yields exec_time_ns under axon."""
    try:
        import antenv.axon_hooks  # noqa: F401
        return
    except ImportError:
        pass
    try:
        from trn_agent_boot.trn_boot import _ntff_profile_via_ctypes

        hook = _ntff_profile_via_ctypes("/opt/axon/libaxon_pjrt.so")
        m = types.ModuleType("antenv.axon_hooks")
        m.get_axon_ntff_profile_hook = lambda: hook
        m.set_axon_ntff_profile_hook = lambda h: None
        sys.modules["antenv.axon_hooks"] = m
        import concourse.bass_utils as bu

        bu.upload_artifacts = lambda tmpdir: f"local:{tmpdir}"
    except Exception:
        pass


def _legalize_waits(nc, max_waits=1):
    """Walrus codegen on this toolchain rejects instructions carrying more
    than a few semaphore waits. Hoist excess waits onto NoOps inserted
    immediately before the offending instruction on the same engine."""
    n_split = 0
    for fn in nc.m.functions:
        for bb in fn.blocks:
            new_list = []
            for ins in bb.instructions:
                si = ins.sync_info
                if si is not None and si.on_wait and len(si.on_wait) > max_waits:
                    waits = list(si.on_wait)
                    while len(waits) > max_waits:
                        chunk, waits = waits[:max_waits], waits[max_waits:]
                        nop = mybir.InstNoOp(
                            name=nc.get_next_instruction_name(),
                            engine=ins.engine,
                            sync_info=mybir.SyncInfo(on_wait=chunk, on_update=[]),
                            bass_nofuse=True,
                        )
                        nc.register_instruction(nop)
                        new_list.append(nop)
                        n_split += 1
                    si.on_wait = waits
                new_list.append(ins)
            bb.instructions[:] = new_list
    return n_split


def build_program():
    nc = bass.Bass()
    xT = nc.declare_dram_parameter("xT", [DIN, TOK_C], BF16, isOutput=False)
    # W_eff.T pre-arranged per output chunk: [n, p, kc, o] with 8KB
    # contiguous per partition line for fat DMA descriptors.
    WTn = nc.declare_dram_parameter(
        "WTn", [NC_OUT, 128, KC, 128], BF16, isOutput=False
    )
    biasv = nc.declare_dram_parameter("biasv", [128, NC_OUT], F32, isOutput=False)
    outT = nc.declare_dram_parameter("outT", [DOUT, TOK_C], F32, isOutput=True)

    PRO_N = 4  # n-groups folded into the k-major prologue
    NQ = 4  # W quarter-tiles per prologue group
    QK = KC // NQ  # k-chunks per quarter

    with tile.TileContext(nc) as tc:
        with (
            tc.tile_pool(name="xpool", bufs=KC + 4) as xpool,
            tc.tile_pool(name="bpool", bufs=2) as bpool,
            tc.tile_pool(name="wqpool", bufs=PRO_N * NQ) as wqpool,
            tc.tile_pool(name="wpool", bufs=3) as wpool,
            tc.tile_pool(name="opool", bufs=2) as opool,
            tc.tile_pool(name="pp", bufs=8, space="PSUM") as pp,
        ):
            # Sync (SP) ring: the x stream as full-row k-chunk tiles, then
            # bias, then (later) the output tiles. Scalar (ACT) ring: W
            # only, so the x stream and outputs never sit behind big W
            # transfers. The 16 SDMA engines are shared between rings at
            # packet granularity and W's 8KB-line packets dominate when
            # both rings have work, so W is fed to the PE in quarter-tiles,
            # quarter-major across the four prologue groups, with the
            # later quarters held behind x-chunk milestones.
            from concourse.tile import add_dep_helper

            # Everything the prologue needs early goes on ONE ring (sync)
            # in priority order — two concurrently-active rings thrash the
            # shared SDMA engines well below single-ring throughput in the
            # startup window. Order: x0-x3 (half-tiles so the very first
            # matmuls unblock ASAP), the four W q0 tiles, then the x tail.
            XSPLIT = 4
            x_full = [None] * KC
            x_half = {}
            x_dmas = [None] * KC
            wq = [[None] * NQ for _ in range(PRO_N)]

            def dma_x(k):
                if k < XSPLIT:
                    for t in range(NT):
                        xh = xpool.tile([128, TBLK], BF16, tag="xh", name=f"x{k}_{t}")
                        dma = nc.sync.dma_start(
                            xh[:],
                            xT[k * 128 : (k + 1) * 128, t * TBLK : (t + 1) * TBLK],
                        )
                        x_half[(k, t)] = xh
                        if t == NT - 1:
                            x_dmas[k] = dma
                else:
                    xt = xpool.tile([128, TOK_C], BF16, tag="xt", name=f"x{k}")
                    x_dmas[k] = nc.sync.dma_start(
                        xt[:], xT[k * 128 : (k + 1) * 128, :]
                    )
                    x_full[k] = xt

            def dma_wq(g, q, engine):
                wt = wqpool.tile([128, QK, 128], BF16, tag="wq", name=f"wq{g}_{q}")
                wdma = engine.dma_start(wt[:], WTn[g, :, q * QK : (q + 1) * QK, :])
                wq[g][q] = wt
                return wdma

            for k in range(XSPLIT):
                dma_x(k)
            for g in range(PRO_N):
                dma_wq(g, 0, nc.scalar)
            for k in range(XSPLIT, KC):
                dma_x(k)

            bt = bpool.tile([128, NC_OUT], F32, name="biasv")
            nc.sync.dma_start(bt[:], biasv[:])

            # Later W quarter batches ride the scalar ring, gated behind
            # x-chunk milestones so they only contend with the x tail.
            gate_x = {1: 8, 2: 16, 3: 24}
            for q in range(1, NQ):
                for g in range(PRO_N):
                    wdma = dma_wq(g, q, nc.scalar)
                    if g == 0:
                        add_dep_helper(
                            wdma.ins,
                            x_dmas[gate_x[q]].ins,
                            reason="hold W quarter batch behind the x stream",
                        )

            pros = {
                (g, t): pp.tile([128, TBLK], F32, tag="ps", name=f"ps{g}_{t}")
                for g in range(PRO_N)
                for t in range(NT)
            }

            wts = {}

            def mm(ps, n, k, t):
                if k < XSPLIT:
                    x_ap = x_half[(k, t)][:]
                else:
                    x_ap = x_full[k][:, t * TBLK : (t + 1) * TBLK]
                if n < PRO_N:
                    w_ap = wq[n][k // QK][:, k % QK, :]
                else:
                    w_ap = wts[n][:, k, :]
                nc.tensor.matmul(
                    ps[:],
                    w_ap,
                    x_ap,
                    start=(k == 0),
                    stop=(k == KC - 1),
                )

            # Warm-up: the HAM clock gate needs ~3.5us of sustained PE
            # activity before it ungates the full 2.4 GHz clock. Run junk
            # matmuls on a memset tile while the first x/W tiles are still
            # in flight so the real stream starts at full speed. The first
            # real matmul of each group uses start=True, which clears the
            # scribbled PSUM bank.
            junk = bpool.tile([128, TBLK], BF16, name="junk")
            nc.vector.memset(junk[:], 0.0)
            for _ in range(30):
                nc.tensor.matmul(
                    pros[(0, 0)][:],
                    junk[:, 0:128],
                    junk[:],
                    start=True,
                    stop=True,
                    skip_group_check=True,
                )

            # k-major prologue: all four groups advance together, so each
            # arriving x chunk feeds 8 matmuls and the PE stays behind the
            # stream. t-outer for the split chunks so the t=0 halves can
            # start before the t=1 halves land.
            for k in range(KC):
                for t in range(NT):
                    for g in range(PRO_N):
                        mm(pros[(g, t)], g, k, t)

            def finish_group(n, ps_map, sub=1):
                ns = slice(n * 128, (n + 1) * 128)
                ot = opool.tile([128, TOK_C], F32, tag="ot", name=f"ot{n}")
                for t in range(NT):
                    # Per-piece drain+DMA: the t=0 half ships while t=1
                    # computes; the last group drains in quarters to
                    # shrink the end-of-kernel serial chain.
                    w = TBLK // sub
                    for s in range(sub):
                        ss = slice(t * TBLK + s * w, t * TBLK + (s + 1) * w)
                        ps_s = ps_map[t][:, s * w : (s + 1) * w]
                        nc.vector.tensor_scalar_add(
                            ot[:, ss], ps_s, bt[:, n : n + 1]
                        )
                        nc.sync.dma_start(outT[ns, ss], ot[:, ss])

            for g in range(PRO_N):
                finish_group(g, {t: pros[(g, t)] for t in range(NT)})

            # Steady state: one output-feature chunk at a time, k-inner.
            for n in range(PRO_N, NC_OUT):
                wts[n] = wpool.tile([128, KC, 128], BF16, tag="wt", name=f"wt{n}")
                wdma = nc.scalar.dma_start(wts[n][:], WTn[n])
                if n == PRO_N:
                    add_dep_helper(
                        wdma.ins,
                        x_dmas[26].ins,
                        reason="hold first steady W chunk behind the x stream",
                    )
                ps_map = {}
                for t in range(NT):
                    ps = pp.tile([128, TBLK], F32, tag="ps", name=f"ps{n}_{t}")
                    ps_map[t] = ps
                    for k in range(KC):
                        mm(ps, n, k, t)
                finish_group(n, ps_map)

    _legalize_waits(nc)
    return nc


_PROGRAM = None


def _get_program():
    global _PROGRAM
    if _PROGRAM is None:
        _PROGRAM = build_program()
    return _PROGRAM


def prepare_in_maps(x, W, bias, A, B):
    x = np.ascontiguousarray(np.asarray(x, dtype=np.float32))
    W = np.asarray(W, dtype=np.float32)
    bias = np.asarray(bias, dtype=np.float32)
    A = np.asarray(A, dtype=np.float32)
    B = np.asarray(B, dtype=np.float32)

    # Merged-LoRA weights: W_eff = W + scale * B @ A (rank-16, cheap).
    W_eff = W + (B * np.float32(SCALE)) @ A
    WT = W_eff.T.astype(BF16_NP)  # [DIN, DOUT]
    # [n, p, kc, o]: chunk-contiguous per partition for 8KB descriptors.
    WTn = np.ascontiguousarray(
        WT.reshape(KC, 128, NC_OUT, 128).transpose(2, 1, 0, 3)
    )
    biasv = np.ascontiguousarray(bias.reshape(NC_OUT, 128).T)

    xf = x.reshape(TOK, DIN)
    in_maps = []
    for c in range(N_CORES):
        xT_c = np.ascontiguousarray(
            xf[c * TOK_C : (c + 1) * TOK_C, :].T.astype(BF16_NP)
        )
        in_maps.append({"xT": xT_c, "WTn": WTn, "biasv": biasv})
    return in_maps


def run(x, W, bias, A, B, trace=False):
    """Returns (out [4,2048,4096], BassKernelResults)."""
    _install_ntff_hook()
    from concourse.bass_utils import run_bass_kernel_spmd

    nc = _get_program()
    in_maps = prepare_in_maps(x, W, bias, A, B)
    res = run_bass_kernel_spmd(
        nc, in_maps, core_ids=list(range(N_CORES)), trace=trace
    )
    shards = [res.results[c]["outT"].T for c in range(N_CORES)]
    out = np.concatenate(shards, axis=0).reshape(B_BATCH, SEQ, DOUT)
    return np.ascontiguousarray(out), res


def kernel(x, W, bias, A, B):
    out, _ = run(x, W, bias, A, B, trace=False)
    return out


if __name__ == "__main__":
    rng = np.random.default_rng(0)
    x = rng.standard_normal((B_BATCH, SEQ, DIN), dtype=np.float32)
    W = rng.standard_normal((DOUT, DIN), dtype=np.float32) * 0.02
    bias = rng.standard_normal(DOUT, dtype=np.float32) * 0.02
    A = rng.standard_normal((RANK, DIN), dtype=np.float32) / RANK
    Bm = rng.standard_normal((DOUT, RANK), dtype=np.float32) * 0.02
    out, res = run(x, W, bias, A, Bm, trace=True)
    ref = x.reshape(TOK, DIN) @ W.T + bias + (x.reshape(TOK, DIN) @ A.T) @ Bm.T * SCALE
    ref = ref.reshape(B_BATCH, SEQ, DOUT)
    err = np.abs(out - ref).max() / np.abs(ref).max()
    print("rel err:", err)
    print("exec_time_ns:", res.exec_time_ns)


# revision 27
# speedup vs baseline: 1.0050x; 1.0050x over previous
"""LoRA linear kernel for Trainium2, SPMD across 8 NeuronCores.

Computes out = x @ W.T + bias + (x @ A.T) @ B.T * (alpha/rank) for
x:[4,2048,4096], W:[4096,4096], bias:[4096], A:[16,4096], B:[4096,16].

The rank-16 LoRA delta is folded on the host (W_eff = W + scale*B@A, the
standard merged-LoRA inference form), so the device runs a single dense
GEMM + bias. Sharding: data-parallel over tokens. Each core takes 1024
tokens and computes all 4096 output features. The host pre-transposes x
and W_eff so the contraction dim lands on the SBUF partition axis; each
core computes out.T for its token shard and the host transposes back.

Operands are bf16 (fp32 PSUM accumulation): unlike fp32r, bf16 matmuls
take a separate LDWEIGHTS that the PE's reorder window hoists behind the
previous matmul's streaming, so the stationary-load cost vanishes. Bias
is added by the DVE during the PSUM->SBUF drain (per-partition scalar).

DMA layout is tuned for descriptor size (per-SDMA-engine throughput is
overhead-bound for small lines): W_eff is pre-arranged on the host so
each 128-output-feature chunk is contiguous per partition (8KB
descriptors on the scalar ring); x streams as full-row k-chunk tiles
(2KB descriptors) on the sync ring; outputs also go out on the sync
ring, which is idle once x has landed, so they never queue behind W.
A 4-group staggered k-major prologue keeps the PE fed while x streams.
"""

import sys
import types

import numpy as np

_REPO = "/opt/trn_rl_repo"
if _REPO not in sys.path:
    sys.path.insert(0, _REPO)

import ml_dtypes  # noqa: E402

import concourse.bass as bass  # noqa: E402
import concourse.mybir as mybir  # noqa: E402
import concourse.tile as tile  # noqa: E402

F32 = mybir.dt.float32
BF16 = mybir.dt.bfloat16
BF16_NP = ml_dtypes.bfloat16

B_BATCH, SEQ, DIN = 4, 2048, 4096
DOUT = 4096
RANK = 16
SCALE = 1.0 / 16.0
N_CORES = 8
TOK = B_BATCH * SEQ  # 8192
TOK_C = TOK // N_CORES  # 1024 tokens per core
KC = DIN // 128  # 32 contraction chunks
NC_OUT = DOUT // 128  # 32 output-feature chunks per core
TBLK = 512  # moving free dim per matmul (one PSUM bank)
NT = TOK_C // TBLK  # 2 token blocks per core


def _install_ntff_hook():
    """Best-effort shim so trace=True yields exec_time_ns under axon."""
    try:
        import antenv.axon_hooks  # noqa: F401
        return
    except ImportError:
        pass
    try:
        from trn_agent_boot.trn_boot import _ntff_profile_via_ctypes

        hook = _ntff_profile_via_ctypes("/opt/axon/libaxon_pjrt.so")
        m = types.ModuleType("antenv.axon_hooks")
        m.get_axon_ntff_profile_hook = lambda: hook
        m.set_axon_ntff_profile_hook = lambda h: None
        sys.modules["antenv.axon_hooks"] = m
        import concourse.bass_utils as bu

        bu.upload_artifacts = lambda tmpdir: f"local:{tmpdir}"
    except Exception:
        pass


def _legalize_waits(nc, max_waits=1):
    """Walrus codegen on this toolchain rejects instructions carrying more
    than a few semaphore waits. Hoist excess waits onto NoOps inserted
    immediately before the offending instruction on the same engine."""
    n_split = 0
    for fn in nc.m.functions:
        for bb in fn.blocks:
            new_list = []
            for ins in bb.instructions:
                si = ins.sync_info
                if si is not None and si.on_wait and len(si.on_wait) > max_waits:
                    waits = list(si.on_wait)
                    while len(waits) > max_waits:
                        chunk, waits = waits[:max_waits], waits[max_waits:]
                        nop = mybir.InstNoOp(
                            name=nc.get_next_instruction_name(),
                            engine=ins.engine,
                            sync_info=mybir.SyncInfo(on_wait=chunk, on_update=[]),
                            bass_nofuse=True,
                        )
                        nc.register_instruction(nop)
                        new_list.append(nop)
                        n_split += 1
                    si.on_wait = waits
                new_list.append(ins)
            bb.instructions[:] = new_list
    return n_split


def build_program():
    nc = bass.Bass()
    xT = nc.declare_dram_parameter("xT", [DIN, TOK_C], BF16, isOutput=False)
    # W_eff.T pre-arranged per output chunk: [n, p, kc, o] with 8KB
    # contiguous per partition line for fat DMA descriptors.
    WTn = nc.declare_dram_parameter(
        "WTn", [NC_OUT, 128, KC, 128], BF16, isOutput=False
    )
    biasv = nc.declare_dram_parameter("biasv", [128, NC_OUT], F32, isOutput=False)
    outT = nc.declare_dram_parameter("outT", [DOUT, TOK_C], F32, isOutput=True)

    PRO_N = 4  # n-groups folded into the k-major prologue
    NQ = 4  # W quarter-tiles per prologue group
    QK = KC // NQ  # k-chunks per quarter

    with tile.TileContext(nc) as tc:
        with (
            tc.tile_pool(name="xpool", bufs=KC + 4) as xpool,
            tc.tile_pool(name="bpool", bufs=2) as bpool,
            tc.tile_pool(name="wqpool", bufs=PRO_N * NQ) as wqpool,
            tc.tile_pool(name="wpool", bufs=3) as wpool,
            tc.tile_pool(name="opool", bufs=2) as opool,
            tc.tile_pool(name="pp", bufs=8, space="PSUM") as pp,
        ):
            # Sync (SP) ring: the x stream as full-row k-chunk tiles, then
            # bias, then (later) the output tiles. Scalar (ACT) ring: W
            # only, so the x stream and outputs never sit behind big W
            # transfers. The 16 SDMA engines are shared between rings at
            # packet granularity and W's 8KB-line packets dominate when
            # both rings have work, so W is fed to the PE in quarter-tiles,
            # quarter-major across the four prologue groups, with the
            # later quarters held behind x-chunk milestones.
            from concourse.tile import add_dep_helper

            # Everything the prologue needs early goes on ONE ring (sync)
            # in priority order — two concurrently-active rings thrash the
            # shared SDMA engines well below single-ring throughput in the
            # startup window. Order: x0-x3 (half-tiles so the very first
            # matmuls unblock ASAP), the four W q0 tiles, then the x tail.
            XSPLIT = 0
            x_full = [None] * KC
            x_half = {}
            x_dmas = [None] * KC
            wq = [[None] * NQ for _ in range(PRO_N)]

            def dma_x(k):
                if k < XSPLIT:
                    for t in range(NT):
                        xh = xpool.tile([128, TBLK], BF16, tag="xh", name=f"x{k}_{t}")
                        dma = nc.sync.dma_start(
                            xh[:],
                            xT[k * 128 : (k + 1) * 128, t * TBLK : (t + 1) * TBLK],
                        )
                        x_half[(k, t)] = xh
                        if t == NT - 1:
                            x_dmas[k] = dma
                else:
                    xt = xpool.tile([128, TOK_C], BF16, tag="xt", name=f"x{k}")
                    x_dmas[k] = nc.sync.dma_start(
                        xt[:], xT[k * 128 : (k + 1) * 128, :]
                    )
                    x_full[k] = xt

            def dma_wq(g, q, engine):
                wt = wqpool.tile([128, QK, 128], BF16, tag="wq", name=f"wq{g}_{q}")
                wdma = engine.dma_start(wt[:], WTn[g, :, q * QK : (q + 1) * QK, :])
                wq[g][q] = wt
                return wdma

            for k in range(XSPLIT):
                dma_x(k)
            for g in range(PRO_N):
                dma_wq(g, 0, nc.scalar)
            for k in range(XSPLIT, KC):
                dma_x(k)

            bt = bpool.tile([128, NC_OUT], F32, name="biasv")
            nc.sync.dma_start(bt[:], biasv[:])

            # Later W quarter batches ride the scalar ring, gated behind
            # x-chunk milestones so they only contend with the x tail.
            gate_x = {1: 8, 2: 16, 3: 24}
            for q in range(1, NQ):
                for g in range(PRO_N):
                    wdma = dma_wq(g, q, nc.scalar)
                    if g == 0:
                        add_dep_helper(
                            wdma.ins,
                            x_dmas[gate_x[q]].ins,
                            reason="hold W quarter batch behind the x stream",
                        )

            pros = {
                (g, t): pp.tile([128, TBLK], F32, tag="ps", name=f"ps{g}_{t}")
                for g in range(PRO_N)
                for t in range(NT)
            }

            wts = {}

            def mm(ps, n, k, t):
                if k < XSPLIT:
                    x_ap = x_half[(k, t)][:]
                else:
                    x_ap = x_full[k][:, t * TBLK : (t + 1) * TBLK]
                if n < PRO_N:
                    w_ap = wq[n][k // QK][:, k % QK, :]
                else:
                    w_ap = wts[n][:, k, :]
                nc.tensor.matmul(
                    ps[:],
                    w_ap,
                    x_ap,
                    start=(k == 0),
                    stop=(k == KC - 1),
                )

            # Warm-up: the HAM clock gate needs ~3.5us of sustained PE
            # activity before it ungates the full 2.4 GHz clock. Run junk
            # matmuls on a memset tile while the first x/W tiles are still
            # in flight so the real stream starts at full speed. The first
            # real matmul of each group uses start=True, which clears the
            # scribbled PSUM bank.
            junk = bpool.tile([128, TBLK], BF16, name="junk")
            nc.vector.memset(junk[:], 0.0)
            for _ in range(24):
                nc.tensor.matmul(
                    pros[(0, 0)][:],
                    junk[:, 0:128],
                    junk[:],
                    start=True,
                    stop=True,
                    skip_group_check=True,
                )

            # k-major prologue: all four groups advance together, so each
            # arriving x chunk feeds 8 matmuls and the PE stays behind the
            # stream. t-outer for the split chunks so the t=0 halves can
            # start before the t=1 halves land.
            for k in range(KC):
                for t in range(NT):
                    for g in range(PRO_N):
                        mm(pros[(g, t)], g, k, t)

            def finish_group(n, ps_map, sub=1):
                ns = slice(n * 128, (n + 1) * 128)
                ot = opool.tile([128, TOK_C], F32, tag="ot", name=f"ot{n}")
                for t in range(NT):
                    # Per-piece drain+DMA: the t=0 half ships while t=1
                    # computes; the last group drains in quarters to
                    # shrink the end-of-kernel serial chain.
                    w = TBLK // sub
                    for s in range(sub):
                        ss = slice(t * TBLK + s * w, t * TBLK + (s + 1) * w)
                        ps_s = ps_map[t][:, s * w : (s + 1) * w]
                        nc.vector.tensor_scalar_add(
                            ot[:, ss], ps_s, bt[:, n : n + 1]
                        )
                        nc.sync.dma_start(outT[ns, ss], ot[:, ss])

            for g in range(PRO_N):
                finish_group(g, {t: pros[(g, t)] for t in range(NT)})

            # Steady state: one output-feature chunk at a time, k-inner.
            for n in range(PRO_N, NC_OUT):
                wts[n] = wpool.tile([128, KC, 128], BF16, tag="wt", name=f"wt{n}")
                wdma = nc.scalar.dma_start(wts[n][:], WTn[n])
                if n == PRO_N:
                    add_dep_helper(
                        wdma.ins,
                        x_dmas[26].ins,
                        reason="hold first steady W chunk behind the x stream",
                    )
                ps_map = {}
                for t in range(NT):
                    ps = pp.tile([128, TBLK], F32, tag="ps", name=f"ps{n}_{t}")
                    ps_map[t] = ps
                    for k in range(KC):
                        mm(ps, n, k, t)
                finish_group(n, ps_map)

    _legalize_waits(nc)
    return nc


_PROGRAM = None


def _get_program():
    global _PROGRAM
    if _PROGRAM is None:
        _PROGRAM = build_program()
    return _PROGRAM


def prepare_in_maps(x, W, bias, A, B):
    x = np.ascontiguousarray(np.asarray(x, dtype=np.float32))
    W = np.asarray(W, dtype=np.float32)
    bias = np.asarray(bias, dtype=np.float32)
    A = np.asarray(A, dtype=np.float32)
    B = np.asarray(B, dtype=np.float32)

    # Merged-LoRA weights: W_eff = W + scale * B @ A (rank-16, cheap).
    W_eff = W + (B * np.float32(SCALE)) @ A
    WT = W_eff.T.astype(BF16_NP)  # [DIN, DOUT]
    # [n, p, kc, o]: chunk-contiguous per partition for 8KB descriptors.
    WTn = np.ascontiguousarray(
        WT.reshape(KC, 128, NC_OUT, 128).transpose(2, 1, 0, 3)
    )
    biasv = np.ascontiguousarray(bias.reshape(NC_OUT, 128).T)

    xf = x.reshape(TOK, DIN)
    in_maps = []
    for c in range(N_CORES):
        xT_c = np.ascontiguousarray(
            xf[c * TOK_C : (c + 1) * TOK_C, :].T.astype(BF16_NP)
        )
        in_maps.append({"xT": xT_c, "WTn": WTn, "biasv": biasv})
    return in_maps


def run(x, W, bias, A, B, trace=False):
    """Returns (out [4,2048,4096], BassKernelResults)."""
    _install_ntff_hook()
    from concourse.bass_utils import run_bass_kernel_spmd

    nc = _get_program()
    in_maps = prepare_in_maps(x, W, bias, A, B)
    res = run_bass_kernel_spmd(
        nc, in_maps, core_ids=list(range(N_CORES)), trace=trace
    )
    shards = [res.results[c]["outT"].T for c in range(N_CORES)]
    out = np.concatenate(shards, axis=0).reshape(B_BATCH, SEQ, DOUT)
    return np.ascontiguousarray(out), res


def kernel(x, W, bias, A, B):
    out, _ = run(x, W, bias, A, B, trace=False)
    return out


if __name__ == "__main__":
    rng = np.random.default_rng(0)
    x = rng.standard_normal((B_BATCH, SEQ, DIN), dtype=np.float32)
    W = rng.standard_normal((DOUT, DIN), dtype=np.float32) * 0.02
    bias = rng.standard_normal(DOUT, dtype=np.float32) * 0.02
    A = rng.standard_normal((RANK, DIN), dtype=np.float32) / RANK
    Bm = rng.standard_normal((DOUT, RANK), dtype=np.float32) * 0.02
    out, res = run(x, W, bias, A, Bm, trace=True)
    ref = x.reshape(TOK, DIN) @ W.T + bias + (x.reshape(TOK, DIN) @ A.T) @ Bm.T * SCALE
    ref = ref.reshape(B_BATCH, SEQ, DOUT)
    err = np.abs(out - ref).max() / np.abs(ref).max()
    print("rel err:", err)
    print("exec_time_ns:", res.exec_time_ns)


# revision 30
# speedup vs baseline: 1.0085x; 1.0036x over previous
"""LoRA linear kernel for Trainium2, SPMD across 8 NeuronCores.

Computes out = x @ W.T + bias + (x @ A.T) @ B.T * (alpha/rank) for
x:[4,2048,4096], W:[4096,4096], bias:[4096], A:[16,4096], B:[4096,16].

The rank-16 LoRA delta is folded on the host (W_eff = W + scale*B@A, the
standard merged-LoRA inference form), so the device runs a single dense
GEMM + bias. Sharding: data-parallel over tokens. Each core takes 1024
tokens and computes all 4096 output features. The host pre-transposes x
and W_eff so the contraction dim lands on the SBUF partition axis; each
core computes out.T for its token shard and the host transposes back.

Operands are bf16 (fp32 PSUM accumulation): unlike fp32r, bf16 matmuls
take a separate LDWEIGHTS that the PE's reorder window hoists behind the
previous matmul's streaming, so the stationary-load cost vanishes. Bias
is added by the DVE during the PSUM->SBUF drain (per-partition scalar).

DMA layout is tuned for descriptor size (per-SDMA-engine throughput is
overhead-bound for small lines): W_eff is pre-arranged on the host so
each 128-output-feature chunk is contiguous per partition (8KB
descriptors on the scalar ring); x streams as full-row k-chunk tiles
(2KB descriptors) on the sync ring; outputs also go out on the sync
ring, which is idle once x has landed, so they never queue behind W.
A 4-group staggered k-major prologue keeps the PE fed while x streams.
"""

import sys
import types

import numpy as np

_REPO = "/opt/trn_rl_repo"
if _REPO not in sys.path:
    sys.path.insert(0, _REPO)

import ml_dtypes  # noqa: E402

import concourse.bass as bass  # noqa: E402
import concourse.mybir as mybir  # noqa: E402
import concourse.tile as tile  # noqa: E402

F32 = mybir.dt.float32
BF16 = mybir.dt.bfloat16
BF16_NP = ml_dtypes.bfloat16

B_BATCH, SEQ, DIN = 4, 2048, 4096
DOUT = 4096
RANK = 16
SCALE = 1.0 / 16.0
N_CORES = 8
TOK = B_BATCH * SEQ  # 8192
TOK_C = TOK // N_CORES  # 1024 tokens per core
KC = DIN // 128  # 32 contraction chunks
NC_OUT = DOUT // 128  # 32 output-feature chunks per core
TBLK = 512  # moving free dim per matmul (one PSUM bank)
NT = TOK_C // TBLK  # 2 token blocks per core


def _install_ntff_hook():
    """Best-effort shim so trace=True yields exec_time_ns under axon."""
    try:
        import antenv.axon_hooks  # noqa: F401
        return
    except ImportError:
        pass
    try:
        from trn_agent_boot.trn_boot import _ntff_profile_via_ctypes

        hook = _ntff_profile_via_ctypes("/opt/axon/libaxon_pjrt.so")
        m = types.ModuleType("antenv.axon_hooks")
        m.get_axon_ntff_profile_hook = lambda: hook
        m.set_axon_ntff_profile_hook = lambda h: None
        sys.modules["antenv.axon_hooks"] = m
        import concourse.bass_utils as bu

        bu.upload_artifacts = lambda tmpdir: f"local:{tmpdir}"
    except Exception:
        pass


def _legalize_waits(nc, max_waits=1):
    """Walrus codegen on this toolchain rejects instructions carrying more
    than a few semaphore waits. Hoist excess waits onto NoOps inserted
    immediately before the offending instruction on the same engine."""
    n_split = 0
    for fn in nc.m.functions:
        for bb in fn.blocks:
            new_list = []
            for ins in bb.instructions:
                si = ins.sync_info
                if si is not None and si.on_wait and len(si.on_wait) > max_waits:
                    waits = list(si.on_wait)
                    while len(waits) > max_waits:
                        chunk, waits = waits[:max_waits], waits[max_waits:]
                        nop = mybir.InstNoOp(
                            name=nc.get_next_instruction_name(),
                            engine=ins.engine,
                            sync_info=mybir.SyncInfo(on_wait=chunk, on_update=[]),
                            bass_nofuse=True,
                        )
                        nc.register_instruction(nop)
                        new_list.append(nop)
                        n_split += 1
                    si.on_wait = waits
                new_list.append(ins)
            bb.instructions[:] = new_list
    return n_split


def build_program():
    nc = bass.Bass()
    xT = nc.declare_dram_parameter("xT", [DIN, TOK_C], BF16, isOutput=False)
    # W_eff.T pre-arranged per output chunk: [n, p, kc, o] with 8KB
    # contiguous per partition line for fat DMA descriptors.
    WTn = nc.declare_dram_parameter(
        "WTn", [NC_OUT, 128, KC, 128], BF16, isOutput=False
    )
    biasv = nc.declare_dram_parameter("biasv", [128, NC_OUT], F32, isOutput=False)
    outT = nc.declare_dram_parameter("outT", [DOUT, TOK_C], F32, isOutput=True)

    PRO_N = 4  # n-groups folded into the k-major prologue
    NQ = 4  # W quarter-tiles per prologue group
    QK = KC // NQ  # k-chunks per quarter

    with tile.TileContext(nc) as tc:
        with (
            tc.tile_pool(name="xpool", bufs=KC + 4) as xpool,
            tc.tile_pool(name="bpool", bufs=2) as bpool,
            tc.tile_pool(name="wqpool", bufs=PRO_N * NQ) as wqpool,
            tc.tile_pool(name="wpool", bufs=3) as wpool,
            tc.tile_pool(name="opool", bufs=2) as opool,
            tc.tile_pool(name="pp", bufs=8, space="PSUM") as pp,
        ):
            # Sync (SP) ring: the x stream as full-row k-chunk tiles, then
            # bias, then (later) the output tiles. Scalar (ACT) ring: W
            # only, so the x stream and outputs never sit behind big W
            # transfers. The 16 SDMA engines are shared between rings at
            # packet granularity and W's 8KB-line packets dominate when
            # both rings have work, so W is fed to the PE in quarter-tiles,
            # quarter-major across the four prologue groups, with the
            # later quarters held behind x-chunk milestones.
            from concourse.tile import add_dep_helper

            # Everything the prologue needs early goes on ONE ring (sync)
            # in priority order — two concurrently-active rings thrash the
            # shared SDMA engines well below single-ring throughput in the
            # startup window. Order: x0-x3 (half-tiles so the very first
            # matmuls unblock ASAP), the four W q0 tiles, then the x tail.
            XSPLIT = 4
            x_full = [None] * KC
            x_half = {}
            x_dmas = [None] * KC
            wq = [[None] * NQ for _ in range(PRO_N)]

            def dma_x(k):
                if k < XSPLIT:
                    for t in range(NT):
                        xh = xpool.tile([128, TBLK], BF16, tag="xh", name=f"x{k}_{t}")
                        dma = nc.sync.dma_start(
                            xh[:],
                            xT[k * 128 : (k + 1) * 128, t * TBLK : (t + 1) * TBLK],
                        )
                        x_half[(k, t)] = xh
                        if t == NT - 1:
                            x_dmas[k] = dma
                else:
                    xt = xpool.tile([128, TOK_C], BF16, tag="xt", name=f"x{k}")
                    x_dmas[k] = nc.sync.dma_start(
                        xt[:], xT[k * 128 : (k + 1) * 128, :]
                    )
                    x_full[k] = xt

            def dma_wq(g, q, engine):
                wt = wqpool.tile([128, QK, 128], BF16, tag="wq", name=f"wq{g}_{q}")
                wdma = engine.dma_start(wt[:], WTn[g, :, q * QK : (q + 1) * QK, :])
                wq[g][q] = wt
                return wdma

            for k in range(XSPLIT):
                dma_x(k)
            for g in range(PRO_N):
                dma_wq(g, 0, nc.scalar)
            for k in range(XSPLIT, KC):
                dma_x(k)

            bt = bpool.tile([128, NC_OUT], F32, name="biasv")
            nc.sync.dma_start(bt[:], biasv[:])

            # Later W quarter batches ride the scalar ring, gated behind
            # x-chunk milestones so they only contend with the x tail.
            gate_x = {1: 8, 2: 16, 3: 24}
            for q in range(1, NQ):
                for g in range(PRO_N):
                    wdma = dma_wq(g, q, nc.scalar)
                    if g == 0:
                        add_dep_helper(
                            wdma.ins,
                            x_dmas[gate_x[q]].ins,
                            reason="hold W quarter batch behind the x stream",
                        )

            pros = {
                (g, t): pp.tile([128, TBLK], F32, tag="ps", name=f"ps{g}_{t}")
                for g in range(PRO_N)
                for t in range(NT)
            }

            wts = {}

            def mm(ps, n, k, t):
                if k < XSPLIT:
                    x_ap = x_half[(k, t)][:]
                else:
                    x_ap = x_full[k][:, t * TBLK : (t + 1) * TBLK]
                if n < PRO_N:
                    w_ap = wq[n][k // QK][:, k % QK, :]
                else:
                    w_ap = wts[n][:, k, :]
                nc.tensor.matmul(
                    ps[:],
                    w_ap,
                    x_ap,
                    start=(k == 0),
                    stop=(k == KC - 1),
                )

            # Warm-up: the HAM clock gate needs ~3.5us of sustained PE
            # activity before it ungates the full 2.4 GHz clock. Run junk
            # matmuls on a memset tile while the first x/W tiles are still
            # in flight so the real stream starts at full speed. The first
            # real matmul of each group uses start=True, which clears the
            # scribbled PSUM bank.
            junk = bpool.tile([128, TBLK], BF16, name="junk")
            nc.vector.memset(junk[:], 0.0)
            for _ in range(30):
                nc.tensor.matmul(
                    pros[(0, 0)][:],
                    junk[:, 0:128],
                    junk[:],
                    start=True,
                    stop=True,
                    skip_group_check=True,
                )

            # k-major prologue: all four groups advance together, so each
            # arriving x chunk feeds 8 matmuls and the PE stays behind the
            # stream. t-outer for the split chunks so the t=0 halves can
            # start before the t=1 halves land.
            for k in range(KC):
                for t in range(NT):
                    for g in range(PRO_N):
                        mm(pros[(g, t)], g, k, t)

            def finish_group(n, ps_map, sub=1):
                ns = slice(n * 128, (n + 1) * 128)
                ot = opool.tile([128, TOK_C], F32, tag="ot", name=f"ot{n}")
                for t in range(NT):
                    # Per-piece drain+DMA: the t=0 half ships while t=1
                    # computes; the last group drains in quarters to
                    # shrink the end-of-kernel serial chain.
                    w = TBLK // sub
                    for s in range(sub):
                        ss = slice(t * TBLK + s * w, t * TBLK + (s + 1) * w)
                        ps_s = ps_map[t][:, s * w : (s + 1) * w]
                        nc.vector.tensor_scalar_add(
                            ot[:, ss], ps_s, bt[:, n : n + 1]
                        )
                        nc.sync.dma_start(outT[ns, ss], ot[:, ss])

            for g in range(PRO_N):
                finish_group(g, {t: pros[(g, t)] for t in range(NT)})

            # Steady state: one output-feature chunk at a time, k-inner.
            for n in range(PRO_N, NC_OUT):
                wts[n] = wpool.tile([128, KC, 128], BF16, tag="wt", name=f"wt{n}")
                wdma = nc.scalar.dma_start(wts[n][:], WTn[n])
                if n == PRO_N:
                    add_dep_helper(
                        wdma.ins,
                        x_dmas[26].ins,
                        reason="hold first steady W chunk behind the x stream",
                    )
                ps_map = {}
                for t in range(NT):
                    ps = pp.tile([128, TBLK], F32, tag="ps", name=f"ps{n}_{t}")
                    ps_map[t] = ps
                    for k in range(KC):
                        mm(ps, n, k, t)
                finish_group(n, ps_map, sub=2 if n == NC_OUT - 1 else 1)

    _legalize_waits(nc)
    return nc


_PROGRAM = None


def _get_program():
    global _PROGRAM
    if _PROGRAM is None:
        _PROGRAM = build_program()
    return _PROGRAM


def prepare_in_maps(x, W, bias, A, B):
    x = np.ascontiguousarray(np.asarray(x, dtype=np.float32))
    W = np.asarray(W, dtype=np.float32)
    bias = np.asarray(bias, dtype=np.float32)
    A = np.asarray(A, dtype=np.float32)
    B = np.asarray(B, dtype=np.float32)

    # Merged-LoRA weights: W_eff = W + scale * B @ A (rank-16, cheap).
    W_eff = W + (B * np.float32(SCALE)) @ A
    WT = W_eff.T.astype(BF16_NP)  # [DIN, DOUT]
    # [n, p, kc, o]: chunk-contiguous per partition for 8KB descriptors.
    WTn = np.ascontiguousarray(
        WT.reshape(KC, 128, NC_OUT, 128).transpose(2, 1, 0, 3)
    )
    biasv = np.ascontiguousarray(bias.reshape(NC_OUT, 128).T)

    xf = x.reshape(TOK, DIN)
    in_maps = []
    for c in range(N_CORES):
        xT_c = np.ascontiguousarray(
            xf[c * TOK_C : (c + 1) * TOK_C, :].T.astype(BF16_NP)
        )
        in_maps.append({"xT": xT_c, "WTn": WTn, "biasv": biasv})
    return in_maps


def run(x, W, bias, A, B, trace=False):
    """Returns (out [4,2048,4096], BassKernelResults)."""
    _install_ntff_hook()
    from concourse.bass_utils import run_bass_kernel_spmd

    nc = _get_program()
    in_maps = prepare_in_maps(x, W, bias, A, B)
    res = run_bass_kernel_spmd(
        nc, in_maps, core_ids=list(range(N_CORES)), trace=trace
    )
    shards = [res.results[c]["outT"].T for c in range(N_CORES)]
    out = np.concatenate(shards, axis=0).reshape(B_BATCH, SEQ, DOUT)
    return np.ascontiguousarray(out), res


def kernel(x, W, bias, A, B):
    out, _ = run(x, W, bias, A, B, trace=False)
    return out


if __name__ == "__main__":
    rng = np.random.default_rng(0)
    x = rng.standard_normal((B_BATCH, SEQ, DIN), dtype=np.float32)
    W = rng.standard_normal((DOUT, DIN), dtype=np.float32) * 0.02
    bias = rng.standard_normal(DOUT, dtype=np.float32) * 0.02
    A = rng.standard_normal((RANK, DIN), dtype=np.float32) / RANK
    Bm = rng.standard_normal((DOUT, RANK), dtype=np.float32) * 0.02
    out, res = run(x, W, bias, A, Bm, trace=True)
    ref = x.reshape(TOK, DIN) @ W.T + bias + (x.reshape(TOK, DIN) @ A.T) @ Bm.T * SCALE
    ref = ref.reshape(B_BATCH, SEQ, DOUT)
    err = np.abs(out - ref).max() / np.abs(ref).max()
    print("rel err:", err)
    print("exec_time_ns:", res.exec_time_ns)


# revision 31
# speedup vs baseline: 1.0102x; 1.0016x over previous
"""LoRA linear kernel for Trainium2, SPMD across 8 NeuronCores.

Computes out = x @ W.T + bias + (x @ A.T) @ B.T * (alpha/rank) for
x:[4,2048,4096], W:[4096,4096], bias:[4096], A:[16,4096], B:[4096,16].

The rank-16 LoRA delta is folded on the host (W_eff = W + scale*B@A, the
standard merged-LoRA inference form), so the device runs a single dense
GEMM + bias. Sharding: data-parallel over tokens. Each core takes 1024
tokens and computes all 4096 output features. The host pre-transposes x
and W_eff so the contraction dim lands on the SBUF partition axis; each
core computes out.T for its token shard and the host transposes back.

Operands are bf16 (fp32 PSUM accumulation): unlike fp32r, bf16 matmuls
take a separate LDWEIGHTS that the PE's reorder window hoists behind the
previous matmul's streaming, so the stationary-load cost vanishes. Bias
is added by the DVE during the PSUM->SBUF drain (per-partition scalar).

DMA layout is tuned for descriptor size (per-SDMA-engine throughput is
overhead-bound for small lines): W_eff is pre-arranged on the host so
each 128-output-feature chunk is contiguous per partition (8KB
descriptors on the scalar ring); x streams as full-row k-chunk tiles
(2KB descriptors) on the sync ring; outputs also go out on the sync
ring, which is idle once x has landed, so they never queue behind W.
A 4-group staggered k-major prologue keeps the PE fed while x streams.
"""

import sys
import types

import numpy as np

_REPO = "/opt/trn_rl_repo"
if _REPO not in sys.path:
    sys.path.insert(0, _REPO)

import ml_dtypes  # noqa: E402

import concourse.bass as bass  # noqa: E402
import concourse.mybir as mybir  # noqa: E402
import concourse.tile as tile  # noqa: E402

F32 = mybir.dt.float32
BF16 = mybir.dt.bfloat16
BF16_NP = ml_dtypes.bfloat16

B_BATCH, SEQ, DIN = 4, 2048, 4096
DOUT = 4096
RANK = 16
SCALE = 1.0 / 16.0
N_CORES = 8
TOK = B_BATCH * SEQ  # 8192
TOK_C = TOK // N_CORES  # 1024 tokens per core
KC = DIN // 128  # 32 contraction chunks
NC_OUT = DOUT // 128  # 32 output-feature chunks per core
TBLK = 512  # moving free dim per matmul (one PSUM bank)
NT = TOK_C // TBLK  # 2 token blocks per core


def _install_ntff_hook():
    """Best-effort shim so trace=True yields exec_time_ns under axon."""
    try:
        import antenv.axon_hooks  # noqa: F401
        return
    except ImportError:
        pass
    try:
        from trn_agent_boot.trn_boot import _ntff_profile_via_ctypes

        hook = _ntff_profile_via_ctypes("/opt/axon/libaxon_pjrt.so")
        m = types.ModuleType("antenv.axon_hooks")
        m.get_axon_ntff_profile_hook = lambda: hook
        m.set_axon_ntff_profile_hook = lambda h: None
        sys.modules["antenv.axon_hooks"] = m
        import concourse.bass_utils as bu

        bu.upload_artifacts = lambda tmpdir: f"local:{tmpdir}"
    except Exception:
        pass


def _legalize_waits(nc, max_waits=1):
    """Walrus codegen on this toolchain rejects instructions carrying more
    than a few semaphore waits. Hoist excess waits onto NoOps inserted
    immediately before the offending instruction on the same engine."""
    n_split = 0
    for fn in nc.m.functions:
        for bb in fn.blocks:
            new_list = []
            for ins in bb.instructions:
                si = ins.sync_info
                if si is not None and si.on_wait and len(si.on_wait) > max_waits:
                    waits = list(si.on_wait)
                    while len(waits) > max_waits:
                        chunk, waits = waits[:max_waits], waits[max_waits:]
                        nop = mybir.InstNoOp(
                            name=nc.get_next_instruction_name(),
                            engine=ins.engine,
                            sync_info=mybir.SyncInfo(on_wait=chunk, on_update=[]),
                            bass_nofuse=True,
                        )
                        nc.register_instruction(nop)
                        new_list.append(nop)
                        n_split += 1
                    si.on_wait = waits
                new_list.append(ins)
            bb.instructions[:] = new_list
    return n_split


def build_program():
    nc = bass.Bass()
    xT = nc.declare_dram_parameter("xT", [DIN, TOK_C], BF16, isOutput=False)
    # W_eff.T pre-arranged per output chunk: [n, p, kc, o] with 8KB
    # contiguous per partition line for fat DMA descriptors.
    WTn = nc.declare_dram_parameter(
        "WTn", [NC_OUT, 128, KC, 128], BF16, isOutput=False
    )
    biasv = nc.declare_dram_parameter("biasv", [128, NC_OUT], F32, isOutput=False)
    outT = nc.declare_dram_parameter("outT", [DOUT, TOK_C], F32, isOutput=True)

    PRO_N = 4  # n-groups folded into the k-major prologue
    NQ = 4  # W quarter-tiles per prologue group
    QK = KC // NQ  # k-chunks per quarter

    with tile.TileContext(nc) as tc:
        with (
            tc.tile_pool(name="xpool", bufs=KC + 4) as xpool,
            tc.tile_pool(name="bpool", bufs=2) as bpool,
            tc.tile_pool(name="wqpool", bufs=PRO_N * NQ) as wqpool,
            tc.tile_pool(name="wpool", bufs=3) as wpool,
            tc.tile_pool(name="opool", bufs=2) as opool,
            tc.tile_pool(name="pp", bufs=8, space="PSUM") as pp,
        ):
            # Sync (SP) ring: the x stream as full-row k-chunk tiles, then
            # bias, then (later) the output tiles. Scalar (ACT) ring: W
            # only, so the x stream and outputs never sit behind big W
            # transfers. The 16 SDMA engines are shared between rings at
            # packet granularity and W's 8KB-line packets dominate when
            # both rings have work, so W is fed to the PE in quarter-tiles,
            # quarter-major across the four prologue groups, with the
            # later quarters held behind x-chunk milestones.
            from concourse.tile import add_dep_helper

            # Everything the prologue needs early goes on ONE ring (sync)
            # in priority order — two concurrently-active rings thrash the
            # shared SDMA engines well below single-ring throughput in the
            # startup window. Order: x0-x3 (half-tiles so the very first
            # matmuls unblock ASAP), the four W q0 tiles, then the x tail.
            XSPLIT = 4
            x_full = [None] * KC
            x_half = {}
            x_dmas = [None] * KC
            wq = [[None] * NQ for _ in range(PRO_N)]

            def dma_x(k):
                if k < XSPLIT:
                    for t in range(NT):
                        xh = xpool.tile([128, TBLK], BF16, tag="xh", name=f"x{k}_{t}")
                        dma = nc.sync.dma_start(
                            xh[:],
                            xT[k * 128 : (k + 1) * 128, t * TBLK : (t + 1) * TBLK],
                        )
                        x_half[(k, t)] = xh
                        if t == NT - 1:
                            x_dmas[k] = dma
                else:
                    xt = xpool.tile([128, TOK_C], BF16, tag="xt", name=f"x{k}")
                    x_dmas[k] = nc.sync.dma_start(
                        xt[:], xT[k * 128 : (k + 1) * 128, :]
                    )
                    x_full[k] = xt

            def dma_wq(g, q, engine):
                wt = wqpool.tile([128, QK, 128], BF16, tag="wq", name=f"wq{g}_{q}")
                wdma = engine.dma_start(wt[:], WTn[g, :, q * QK : (q + 1) * QK, :])
                wq[g][q] = wt
                return wdma

            for k in range(XSPLIT):
                dma_x(k)
            for g in range(PRO_N):
                dma_wq(g, 0, nc.scalar)
            for k in range(XSPLIT, KC):
                dma_x(k)

            bt = bpool.tile([128, NC_OUT], F32, name="biasv")
            nc.sync.dma_start(bt[:], biasv[:])

            # Later W quarter batches ride the scalar ring, gated behind
            # x-chunk milestones so they only contend with the x tail.
            gate_x = {1: 8, 2: 16, 3: 24}
            for q in range(1, NQ):
                for g in range(PRO_N):
                    wdma = dma_wq(g, q, nc.scalar)
                    if g == 0:
                        add_dep_helper(
                            wdma.ins,
                            x_dmas[gate_x[q]].ins,
                            reason="hold W quarter batch behind the x stream",
                        )

            pros = {
                (g, t): pp.tile([128, TBLK], F32, tag="ps", name=f"ps{g}_{t}")
                for g in range(PRO_N)
                for t in range(NT)
            }

            wts = {}

            def mm(ps, n, k, t):
                if k < XSPLIT:
                    x_ap = x_half[(k, t)][:]
                else:
                    x_ap = x_full[k][:, t * TBLK : (t + 1) * TBLK]
                if n < PRO_N:
                    w_ap = wq[n][k // QK][:, k % QK, :]
                else:
                    w_ap = wts[n][:, k, :]
                nc.tensor.matmul(
                    ps[:],
                    w_ap,
                    x_ap,
                    start=(k == 0),
                    stop=(k == KC - 1),
                )

            # Warm-up: the HAM clock gate needs ~3.5us of sustained PE
            # activity before it ungates the full 2.4 GHz clock. Run junk
            # matmuls on a memset tile while the first x/W tiles are still
            # in flight so the real stream starts at full speed. The first
            # real matmul of each group uses start=True, which clears the
            # scribbled PSUM bank.
            junk = bpool.tile([128, TBLK], BF16, name="junk")
            nc.vector.memset(junk[:], 0.0)
            for _ in range(30):
                nc.tensor.matmul(
                    pros[(0, 0)][:],
                    junk[:, 0:128],
                    junk[:],
                    start=True,
                    stop=True,
                    skip_group_check=True,
                )

            # k-major prologue: all four groups advance together, so each
            # arriving x chunk feeds 8 matmuls and the PE stays behind the
            # stream. t-outer for the split chunks so the t=0 halves can
            # start before the t=1 halves land.
            for k in range(KC):
                for t in range(NT):
                    for g in range(PRO_N):
                        mm(pros[(g, t)], g, k, t)

            def finish_group(n, ps_map, sub=1):
                ns = slice(n * 128, (n + 1) * 128)
                ot = opool.tile([128, TOK_C], F32, tag="ot", name=f"ot{n}")
                for t in range(NT):
                    # Per-piece drain+DMA: the t=0 half ships while t=1
                    # computes; the last group drains in quarters to
                    # shrink the end-of-kernel serial chain.
                    w = TBLK // sub
                    for s in range(sub):
                        ss = slice(t * TBLK + s * w, t * TBLK + (s + 1) * w)
                        ps_s = ps_map[t][:, s * w : (s + 1) * w]
                        nc.vector.tensor_scalar_add(
                            ot[:, ss], ps_s, bt[:, n : n + 1]
                        )
                        nc.sync.dma_start(outT[ns, ss], ot[:, ss])

            for g in range(PRO_N):
                finish_group(g, {t: pros[(g, t)] for t in range(NT)})

            # Steady state: one output-feature chunk at a time, k-inner.
            for n in range(PRO_N, NC_OUT):
                wts[n] = wpool.tile([128, KC, 128], BF16, tag="wt", name=f"wt{n}")
                wdma = nc.scalar.dma_start(wts[n][:], WTn[n])
                if n == PRO_N:
                    add_dep_helper(
                        wdma.ins,
                        x_dmas[26].ins,
                        reason="hold first steady W chunk behind the x stream",
                    )
                ps_map = {}
                for t in range(NT):
                    ps = pp.tile([128, TBLK], F32, tag="ps", name=f"ps{n}_{t}")
                    ps_map[t] = ps
                    for k in range(KC):
                        mm(ps, n, k, t)
                finish_group(n, ps_map)

    _legalize_waits(nc)
    return nc


_PROGRAM = None


def _get_program():
    global _PROGRAM
    if _PROGRAM is None:
        _PROGRAM = build_program()
    return _PROGRAM


def prepare_in_maps(x, W, bias, A, B):
    x = np.ascontiguousarray(np.asarray(x, dtype=np.float32))
    W = np.asarray(W, dtype=np.float32)
    bias = np.asarray(bias, dtype=np.float32)
    A = np.asarray(A, dtype=np.float32)
    B = np.asarray(B, dtype=np.float32)

    # Merged-LoRA weights: W_eff = W + scale * B @ A (rank-16, cheap).
    W_eff = W + (B * np.float32(SCALE)) @ A
    WT = W_eff.T.astype(BF16_NP)  # [DIN, DOUT]
    # [n, p, kc, o]: chunk-contiguous per partition for 8KB descriptors.
    WTn = np.ascontiguousarray(
        WT.reshape(KC, 128, NC_OUT, 128).transpose(2, 1, 0, 3)
    )
    biasv = np.ascontiguousarray(bias.reshape(NC_OUT, 128).T)

    xf = x.reshape(TOK, DIN)
    in_maps = []
    for c in range(N_CORES):
        xT_c = np.ascontiguousarray(
            xf[c * TOK_C : (c + 1) * TOK_C, :].T.astype(BF16_NP)
        )
        in_maps.append({"xT": xT_c, "WTn": WTn, "biasv": biasv})
    return in_maps


def run(x, W, bias, A, B, trace=False):
    """Returns (out [4,2048,4096], BassKernelResults)."""
    _install_ntff_hook()
    from concourse.bass_utils import run_bass_kernel_spmd

    nc = _get_program()
    in_maps = prepare_in_maps(x, W, bias, A, B)
    res = run_bass_kernel_spmd(
        nc, in_maps, core_ids=list(range(N_CORES)), trace=trace
    )
    shards = [res.results[c]["outT"].T for c in range(N_CORES)]
    out = np.concatenate(shards, axis=0).reshape(B_BATCH, SEQ, DOUT)
    return np.ascontiguousarray(out), res


def kernel(x, W, bias, A, B):
    out, _ = run(x, W, bias, A, B, trace=False)
    return out


if __name__ == "__main__":
    rng = np.random.default_rng(0)
    x = rng.standard_normal((B_BATCH, SEQ, DIN), dtype=np.float32)
    W = rng.standard_normal((DOUT, DIN), dtype=np.float32) * 0.02
    bias = rng.standard_normal(DOUT, dtype=np.float32) * 0.02
    A = rng.standard_normal((RANK, DIN), dtype=np.float32) / RANK
    Bm = rng.standard_normal((DOUT, RANK), dtype=np.float32) * 0.02
    out, res = run(x, W, bias, A, Bm, trace=True)
    ref = x.reshape(TOK, DIN) @ W.T + bias + (x.reshape(TOK, DIN) @ A.T) @ Bm.T * SCALE
    ref = ref.reshape(B_BATCH, SEQ, DOUT)
    err = np.abs(out - ref).max() / np.abs(ref).max()
    print("rel err:", err)
    print("exec_time_ns:", res.exec_time_ns)
